# revision 34
# baseline (speedup 1.0000x reference)
"""Trainium2 Bass kernel for the gnn_message_passing problem, v7.

Strategy (8 NeuronCores, SPMD, fully local — no collectives, no indirect DMA):
  - Each core handles 64 sessions.  The host stages, per core, the raw
    img/txt/emb rows referenced by its own `inputs` (3200 entries, fp8,
    transposed, row-chunk-blocked) and `item` (3200 entries, fp8,
    seg-major row chunks) in exactly the SBUF layout the kernel wants, so
    every DMA is a few contiguous descriptors per partition, split across
    both HWDGE queues.  DMA splits are along non-contraction axes so PE
    chains complete as quarters arrive.
  - h0 path: project input rows with weights-stationary fp8 DoubleRow
    matmuls (Wcat = [0.1*img_W; 0.15*txt_W; I], emb' through the identity
    chunk), PE-transpose to pair-major rows.
  - Session path: masked per-session sums via mask-stationary fp8
    DoubleRow matmuls with N=512 (4 PSUM seg chains held across the rr
    load), then 15 PE transposes give the transposed sums ST; projection
    via bf16 weight chunks with biases as weight rows against an
    indicator column.  2-way softmax as sigmoid of the logit difference.
  - Hypergraph: h' = D_n^-1 H D_e^-1 H^T h + s per pair, precomputed as
    An = transpose(D_n^-1 (H De^-1 H^T)) so the inner loop is just two
    matmuls per pair plus one batched PSUM+srep add per 4-pair group.
    Degrees via two batched DVE reduces.
"""

import sys

sys.path.insert(0, "/opt/trn_rl_repo")

import numpy as np
import ml_dtypes

import concourse.bass as bass
import concourse.bacc as bacc
import concourse.mybir as mybir
import concourse.tile as tile
from concourse import bass_utils

BF16 = ml_dtypes.bfloat16
FP8 = ml_dtypes.float8_e4m3fn


class Cfg:
    def __init__(self):
        self.N = 50000
        self.D = 128
        self.IMG = 1000
        self.TXT = 768
        self.B = 512
        self.L = 50
        self.NC = 8
        self.BS = self.B // self.NC      # 64 sessions per core
        self.NPAIR = self.BS // 2        # 32
        self.L2 = 2 * self.L             # 100
        self.NE = self.BS * self.L       # 3200 entries per core
        self.KP = 15                     # proj k-chunks (img|txt|pad|emb)
        self.KR = 25                     # rr row chunks (3200 = 25*128)
        self.RW = 1920                   # rr width (15*128): emb|img|txt|ind|pad
        self.MR = 15                     # ST col chunks
        self.SEG = 4                     # rr col segments (480 wide)
        self.SW = 480
        self.NW = 17                     # session weight chunks
        # proj row-chunk blocks (entry dim)
        self.RCH = [(i * 512, 512) for i in range(6)] + [(3072, 128)]
        self.RCOFF = [0]
        for _, w in self.RCH:
            self.RCOFF.append(self.RCOFF[-1] + self.KP * w)


REAL = Cfg()


def build_program(c: Cfg):
    f32 = mybir.dt.float32
    b16 = mybir.dt.bfloat16
    f16 = mybir.dt.float16
    f8 = mybir.dt.float8e4
    AF = mybir.ActivationFunctionType
    AX = mybir.AxisListType
    OP = mybir.AluOpType

    nc = bacc.Bacc("TRN2", target_bir_lowering=False, debug=False,
                   num_devices=c.NC)

    def ein(nm, sh, dt):
        return nc.dram_tensor(nm, sh, dt, kind="ExternalInput")

    PRW = c.RCOFF[-1]                    # 15*3200 free elems
    prTd = ein("prTd", [128, PRW], f8)
    rrd = ein("rrd", [128, c.KR * c.RW], f8)   # [p][m][k][128] m-major
    mkTd = ein("mkTd", [128, c.KR * c.BS], f8)
    Wcatd = ein("Wcatd", [128, c.KP * c.D], f8)
    Wsd = ein("Wsd", [128, c.NW * c.D], b16)
    Gd = ein("Gd", [c.L2, c.NPAIR * c.L2], f8)
    GTd = ein("GTd", [c.L2, c.NPAIR * c.L2], f8)
    seld = ein("seld", [c.BS, c.NPAIR * c.L2], b16)
    mbld = ein("mbld", [c.L, c.BS], b16)
    gvW = ein("gvW", [c.D, c.D], f32)
    gvB = ein("gvB", [c.D, 1], f32)
    gtW = ein("gtW", [c.D, c.D], f32)
    gtB = ein("gtB", [c.D, 1], f32)
    q1W = ein("q1W", [c.D, c.D], f32)
    q1B = ein("q1B", [c.D, 1], f32)
    q2W = ein("q2W", [c.D, 1], f32)

    outd = nc.dram_tensor("outd", [c.L2, c.NPAIR * c.D], f16,
                          kind="ExternalOutput")

    IMCH = list(range(1, 9))     # img rows live in ST chunks 1..8
    TXCH = list(range(8, 15))    # txt rows live in ST chunks 8..14
    DR = mybir.MatmulPerfMode.DoubleRow

    with tile.TileContext(nc) as tc:
        with (
            tc.tile_pool(name="wpool", bufs=1) as wp,
            tc.tile_pool(name="cpool", bufs=1) as cb,
            tc.tile_pool(name="cs", bufs=3) as cs,
        ):
            # ---------------- sync queue: projection inputs ----------------
            wc = wp.tile([128, c.KP * c.D], f8, tag="wc")
            nc.sync.dma_start(wc[:], Wcatd[:])
            prT = wp.tile([128, PRW], f8, tag="prT")
            nc.sync.dma_start(prT[:, c.RCOFF[0]:c.RCOFF[1]],
                              prTd[:, c.RCOFF[0]:c.RCOFF[1]])
            Gsb = wp.tile([c.L2, c.NPAIR * c.L2], f8, tag="Gsb")
            nc.sync.dma_start(Gsb[:], Gd[:])
            GTsb = wp.tile([c.L2, c.NPAIR * c.L2], f8, tag="GTsb")
            nc.sync.dma_start(GTsb[:], GTd[:])
            for a, b in [(1, 3), (3, 5), (5, 7)]:
                nc.sync.dma_start(prT[:, c.RCOFF[a]:c.RCOFF[b]],
                                  prTd[:, c.RCOFF[a]:c.RCOFF[b]])

            # ---------------- scalar queue: session inputs ----------------
            mkT = wp.tile([128, c.KR * c.BS], f8, tag="mkT")
            nc.scalar.dma_start(mkT[:], mkTd[:])
            rr = wp.tile([128, c.KR * c.RW], f8, tag="rr")
            MW = c.KR * c.D              # free elems per m block
            for a, b in [(0, 4), (4, 8), (8, 12), (12, 15)]:
                nc.scalar.dma_start(rr[:, a * MW:b * MW],
                                    rrd[:, a * MW:b * MW])
            sel = wp.tile([c.BS, c.NPAIR * c.L2], b16, tag="sel")
            nc.scalar.dma_start(sel[:], seld[:])
            ws = wp.tile([128, c.NW * c.D], b16, tag="ws")
            nc.gpsimd.dma_start(ws[:], Wsd[:])
            mbl = wp.tile([c.L, c.BS], b16, tag="mbl")
            nc.gpsimd.dma_start(mbl[:], mbld[:])
            wgv = cb.tile([c.D, c.D], f32, tag="wgv")
            wgt = cb.tile([c.D, c.D], f32, tag="wgt")
            wq1 = cb.tile([c.D, c.D], f32, tag="wq1")
            wq2 = cb.tile([c.D, 1], f32, tag="wq2")
            bgv = cb.tile([c.D, 1], f32, tag="bgv")
            bgt = cb.tile([c.D, 1], f32, tag="bgt")
            bq1 = cb.tile([c.D, 1], f32, tag="bq1")
            nc.gpsimd.dma_start(wgv[:], gvW[:])
            nc.gpsimd.dma_start(wgt[:], gtW[:])
            nc.gpsimd.dma_start(wq1[:], q1W[:])
            nc.gpsimd.dma_start(wq2[:], q2W[:])
            nc.gpsimd.dma_start(bgv[:], gvB[:])
            nc.gpsimd.dma_start(bgt[:], gtB[:])
            nc.gpsimd.dma_start(bq1[:], q1B[:])

            # ---------------- constants / identities ----------------
            identf = wp.tile([128, 128], f32, tag="idf")
            ident16 = wp.tile([128, 128], b16, tag="id16")
            from concourse.masks import make_identity
            make_identity(nc, identf[:])
            make_identity(nc, ident16[:])
            ones50 = wp.tile([c.L, 1], b16, tag="ones50")
            nc.vector.memset(ones50[:], 1.0)
            onesf = wp.tile([1, c.D], f32, tag="onesf")
            nc.vector.memset(onesf[:], 1.0)

            psA_ctx = tc.tile_pool(name="psA", bufs=2, space="PSUM")
            psA = psA_ctx.__enter__()
            psS_ctx = tc.tile_pool(name="psS", bufs=2, space="PSUM")
            psS = psS_ctx.__enter__()
            psT_ctx = tc.tile_pool(name="psT", bufs=2, space="PSUM")
            psT = psT_ctx.__enter__()

            hTs = cb.tile([128, c.NE], b16, tag="hTs")

            def proj_chain(rc):
                r0, w = c.RCH[rc]
                off = c.RCOFF[rc]
                pa = psA.tile([128, 512], f32, tag="psA", name=f"pa{rc}")
                for k2 in range(7):
                    nc.tensor.matmul(
                        pa[:, 0:w],
                        lhsT=wc[:, 2 * k2 * c.D:(2 * k2 + 2) * c.D].rearrange(
                            "p (k d) -> p k d", k=2),
                        rhs=prT[:, off + 2 * k2 * w:off + (2 * k2 + 2) * w]
                        .rearrange("p (k w) -> p k w", k=2),
                        start=(k2 == 0), stop=False, perf_mode=DR)
                nc.tensor.matmul(
                    pa[:, 0:w], lhsT=wc[:, 14 * c.D:15 * c.D],
                    rhs=prT[:, off + 14 * w:off + 15 * w],
                    start=False, stop=True)
                nc.scalar.copy(hTs[:, r0:r0 + w], pa[:, 0:w])

            ST = cb.tile([128, c.MR * c.BS], b16, tag="ST")
            Semb = cb.tile([128, c.BS], f32, tag="Semb")

            def sum_chain(m):
                ps = psS.tile([128, 512], f32, tag="psS", name=f"ps{m}")
                o = m * c.KR * c.D
                for k2 in range(12):
                    nc.tensor.matmul(
                        ps[:, 0:c.BS],
                        lhsT=rr[:, o + 2 * k2 * c.D:o + (2 * k2 + 2) * c.D]
                        .rearrange("p (k d) -> p k d", k=2),
                        rhs=mkT[:, 2 * k2 * c.BS:(2 * k2 + 2) * c.BS]
                        .rearrange("p (k b) -> p k b", k=2),
                        start=(k2 == 0), stop=False, perf_mode=DR)
                nc.tensor.matmul(
                    ps[:, 0:c.BS], lhsT=rr[:, o + 24 * c.D:o + 25 * c.D],
                    rhs=mkT[:, 24 * c.BS:25 * c.BS], start=False, stop=True)
                if m == 0:
                    nc.vector.tensor_copy(Semb[:], ps[:, 0:c.BS])
                elif m % 2 == 0:
                    nc.vector.tensor_copy(ST[:, m * c.BS:(m + 1) * c.BS],
                                          ps[:, 0:c.BS])
                else:
                    nc.scalar.copy(ST[:, m * c.BS:(m + 1) * c.BS],
                                   ps[:, 0:c.BS])

            # degree reciprocals via batched DVE reduces
            ideA = cb.tile([c.L2, c.NPAIR], f32, tag="ideA")
            idnA = cb.tile([c.L2, c.NPAIR], f32, tag="idnA")
            GTn = cb.tile([c.L2, c.NPAIR * c.L2], b16, tag="GTn")

            ones100 = wp.tile([c.L2, 1], b16, tag="ones100")
            nc.vector.memset(ones100[:], 1.0)

            def deg_all():
                pde = psT.tile([128, 512], f32, tag="psTf", name="pde")
                for p in range(c.NPAIR):
                    nc.tensor.matmul(pde[0:c.L2, p:p + 1],
                                     lhsT=Gsb[:, p * c.L2:(p + 1) * c.L2],
                                     rhs=ones100[:], start=True, stop=True)
                nc.vector.reciprocal(ideA[:], pde[0:c.L2, 0:c.NPAIR])
                pdn = psT.tile([128, 512], f32, tag="psTf", name="pdn")
                for p in range(c.NPAIR):
                    nc.tensor.matmul(pdn[0:c.L2, p:p + 1],
                                     lhsT=GTsb[:, p * c.L2:(p + 1) * c.L2],
                                     rhs=ones100[:], start=True, stop=True)
                nc.vector.reciprocal(idnA[:], pdn[0:c.L2, 0:c.NPAIR])

            def gtn_all():
                # GTn = GT scaled by 1/deg_e along partitions (e)
                for p in range(c.NPAIR):
                    if p % 2 == 0:
                        nc.vector.tensor_scalar_mul(
                            GTn[:, p * c.L2:(p + 1) * c.L2],
                            GTsb[:, p * c.L2:(p + 1) * c.L2],
                            ideA[:, p:p + 1])
                    else:
                        nc.scalar.activation(
                            GTn[:, p * c.L2:(p + 1) * c.L2],
                            GTsb[:, p * c.L2:(p + 1) * c.L2], AF.Copy,
                            scale=ideA[:, p:p + 1])

            h0 = cb.tile([c.L2, c.NPAIR * c.D], b16, tag="h0")

            def transp(p):
                tr = psT.tile([128, 512], b16, tag="psT16", name=f"tr{p}")
                nc.tensor.transpose(tr[0:c.L2, 0:c.D],
                                    hTs[:, p * c.L2:(p + 1) * c.L2],
                                    ident16[:])
                if p % 2 == 0:
                    nc.scalar.copy(h0[:, p * c.D:(p + 1) * c.D],
                                   tr[0:c.L2, 0:c.D])
                else:
                    nc.vector.tensor_copy(h0[:, p * c.D:(p + 1) * c.D],
                                          tr[0:c.L2, 0:c.D])

            # Abar = H De^-1 H^T (symmetric), 4 pairs per PSUM tile
            Asb = cb.tile([c.L2, c.NPAIR * c.L2], b16, tag="Asb")

            def abar(g):
                pa_ = psT.tile([128, 512], f32, tag="psTf", name=f"ab{g}")
                for j in range(4):
                    p = 4 * g + j
                    nc.tensor.matmul(pa_[0:c.L2, j * c.L2:(j + 1) * c.L2],
                                     lhsT=GTsb[:, p * c.L2:(p + 1) * c.L2],
                                     rhs=GTn[:, p * c.L2:(p + 1) * c.L2],
                                     start=True, stop=True)
                if g % 2 == 0:
                    nc.scalar.copy(Asb[:, g * 4 * c.L2:(g + 1) * 4 * c.L2],
                                   pa_[0:c.L2, 0:4 * c.L2])
                else:
                    nc.vector.tensor_copy(
                        Asb[:, g * 4 * c.L2:(g + 1) * 4 * c.L2],
                        pa_[0:c.L2, 0:4 * c.L2])

            # ---- phase-1 PE program, ordered to match DMA arrivals ----
            proj_chain(0)
            sum_chain(0)
            sum_chain(1)
            proj_chain(1)
            sum_chain(2)
            sum_chain(3)
            deg_all()
            proj_chain(2)
            sum_chain(4)
            sum_chain(5)
            gtn_all()
            proj_chain(3)
            sum_chain(6)
            sum_chain(7)
            for g in range(4):
                abar(g)
            proj_chain(4)
            sum_chain(8)
            sum_chain(9)
            for g in range(4, 8):
                abar(g)
            proj_chain(5)
            sum_chain(10)
            sum_chain(11)
            proj_chain(6)
            for p in range(c.NPAIR):
                transp(p)
            sum_chain(12)
            sum_chain(13)
            sum_chain(14)

            # denom
            dT = psS.tile([128, 512], f32, tag="psS", name="dT")
            nc.tensor.matmul(dT[0:1, 0:c.BS], lhsT=ones50[:], rhs=mbl[:],
                             start=True, stop=True)
            invd = cb.tile([1, c.BS], f32, tag="invd")
            nc.vector.reciprocal(invd[:], dT[0:1, 0:c.BS])

            # ---------------- session projections ----------------
            pim = psA.tile([128, 512], f32, tag="psA", name="pim")
            for i, m in enumerate(IMCH):
                nc.tensor.matmul(pim[:, 0:c.BS],
                                 lhsT=ws[:, i * c.D:(i + 1) * c.D],
                                 rhs=ST[:, m * c.BS:(m + 1) * c.BS],
                                 start=(i == 0), stop=False)
            XimQ = cs.tile([c.D, c.BS], f32, tag="XimQ")
            nc.scalar.copy(XimQ[:], pim[:, 0:c.BS])
            nc.tensor.matmul(pim[:, 0:c.BS], lhsT=ws[:, 8 * c.D:9 * c.D],
                             rhs=ST[:, 14 * c.BS:15 * c.BS],
                             start=False, stop=True)
            Xim = cb.tile([c.D, c.BS], f32, tag="Xim")
            nc.vector.tensor_copy(Xim[:], pim[:, 0:c.BS])

            ptx = psA.tile([128, 512], f32, tag="psA", name="ptx")
            for i, m in enumerate(TXCH):
                nc.tensor.matmul(ptx[:, 0:c.BS],
                                 lhsT=ws[:, (9 + i) * c.D:(10 + i) * c.D],
                                 rhs=ST[:, m * c.BS:(m + 1) * c.BS],
                                 start=(i == 0), stop=False)
            XtxQ = cs.tile([c.D, c.BS], f32, tag="XtxQ")
            nc.scalar.copy(XtxQ[:], ptx[:, 0:c.BS])
            nc.tensor.matmul(ptx[:, 0:c.BS], lhsT=ws[:, 16 * c.D:17 * c.D],
                             rhs=ST[:, 14 * c.BS:15 * c.BS],
                             start=False, stop=True)
            Xtx = cb.tile([c.D, c.BS], f32, tag="Xtx")
            nc.vector.tensor_copy(Xtx[:], ptx[:, 0:c.BS])

            # Xit = Semb' + 0.1 Xim' + 0.15 Xtx'
            Xit = cb.tile([c.D, c.BS], f32, tag="Xit")
            nc.vector.scalar_tensor_tensor(Xit[:], XimQ[:], 0.1, Semb[:],
                                           op0=OP.mult, op1=OP.add)
            nc.vector.scalar_tensor_tensor(Xit[:], XtxQ[:], 0.15, Xit[:],
                                           op0=OP.mult, op1=OP.add)

            # ---------------- fusion (transposed [128, 64]) ----------------
            def rep_row(row, nm):
                rp = psA.tile([128, 512], f32, tag="psA", name=nm)
                nc.tensor.matmul(rp[:, 0:c.BS], lhsT=onesf[:], rhs=row,
                                 start=True, stop=True)
                return rp

            Xim_m = cb.tile([c.D, c.BS], f32, tag="Xim_m")
            Xtx_m = cb.tile([c.D, c.BS], f32, tag="Xtx_m")
            Xit_m = cb.tile([c.D, c.BS], f32, tag="Xit_m")
            ir = rep_row(invd[:], "ir")
            nc.vector.tensor_tensor(Xim_m[:], Xim[:], ir[:, 0:c.BS], op=OP.mult)
            nc.vector.tensor_tensor(Xtx_m[:], Xtx[:], ir[:, 0:c.BS], op=OP.mult)
            nc.vector.tensor_tensor(Xit_m[:], Xit[:], ir[:, 0:c.BS], op=OP.mult)

            pgv = psA.tile([128, 512], f32, tag="psA", name="pgv")
            nc.tensor.matmul(pgv[:, 0:c.BS], lhsT=wgv[:], rhs=Xim_m[:],
                             start=True, stop=True)
            gv1 = cs.tile([c.D, c.BS], f32, tag="gv1")
            nc.scalar.activation(gv1[:], pgv[:, 0:c.BS], AF.Sigmoid,
                                 bias=bgv[:, :1], scale=2.0)
            pgt = psA.tile([128, 512], f32, tag="psA", name="pgt")
            nc.tensor.matmul(pgt[:, 0:c.BS], lhsT=wgt[:], rhs=Xtx_m[:],
                             start=True, stop=True)
            gt1 = cs.tile([c.D, c.BS], f32, tag="gt1")
            nc.scalar.activation(gt1[:], pgt[:, 0:c.BS], AF.Sigmoid,
                                 bias=bgt[:, :1], scale=2.0)
            sid = cb.tile([c.D, c.BS], f32, tag="sid")
            std = cb.tile([c.D, c.BS], f32, tag="std")
            nc.vector.tensor_mul(sid[:], Xit_m[:], gv1[:])
            nc.vector.tensor_mul(std[:], Xit_m[:], gt1[:])

            def qc(xin, tag):
                pq = psA.tile([128, 512], f32, tag="psA", name="pq" + tag)
                nc.tensor.matmul(pq[:, 0:c.BS], lhsT=wq1[:], rhs=xin[:],
                                 start=True, stop=True)
                th = cs.tile([c.D, c.BS], f32, tag="th")
                nc.scalar.activation(th[:], pq[:, 0:c.BS], AF.Tanh,
                                     bias=bq1[:, :1], scale=1.0)
                qq = psS.tile([128, 512], f32, tag="psS", name="qq" + tag)
                nc.tensor.matmul(qq[0:1, 0:c.BS], lhsT=wq2[:], rhs=th[:],
                                 start=True, stop=True)
                qv = cs.tile([1, c.BS], f32, tag="qv" + tag)
                nc.vector.tensor_copy(qv[:], qq[0:1, 0:c.BS])
                return qv

            q1v = qc(sid, "a")
            q2v = qc(std, "b")
            # 2-way softmax: w1 = sigmoid(q1-q2), w2 = 1-w1
            qd = cs.tile([1, c.BS], f32, tag="qd")
            nc.vector.tensor_sub(qd[:], q1v[:], q2v[:])
            w1 = cs.tile([1, c.BS], f32, tag="w1")
            nc.scalar.activation(w1[:], qd[:], AF.Sigmoid)
            w1r = rep_row(w1[:], "w1r")
            # com = std + w1*(sid-std)
            com = cb.tile([c.D, c.BS], f32, tag="com")
            nc.vector.tensor_sub(com[:], sid[:], std[:])
            nc.vector.tensor_mul(com[:], com[:], w1r[:, 0:c.BS])
            nc.vector.tensor_add(com[:], com[:], std[:])

            pg2 = psA.tile([128, 512], f32, tag="psA", name="pg2")
            nc.tensor.matmul(pg2[:, 0:c.BS], lhsT=wgv[:], rhs=Xit_m[:],
                             start=True, stop=True)
            gv2 = cs.tile([c.D, c.BS], f32, tag="gv2")
            nc.scalar.activation(gv2[:], pg2[:, 0:c.BS], AF.Sigmoid,
                                 bias=bgv[:, :1], scale=1.0)
            pg3 = psA.tile([128, 512], f32, tag="psA", name="pg3")
            nc.tensor.matmul(pg3[:, 0:c.BS], lhsT=wgt[:], rhs=Xit_m[:],
                             start=True, stop=True)
            gt2 = cs.tile([c.D, c.BS], f32, tag="gt2")
            nc.scalar.activation(gt2[:], pg3[:, 0:c.BS], AF.Sigmoid,
                                 bias=bgt[:, :1], scale=1.0)

            sep = cs.tile([c.D, c.BS], f32, tag="sep")
            nc.vector.tensor_sub(sep[:], sid[:], com[:])
            nc.vector.tensor_mul(sep[:], gv2[:], sep[:])
            sep2 = cs.tile([c.D, c.BS], f32, tag="sep2")
            nc.vector.tensor_sub(sep2[:], std[:], com[:])
            nc.vector.tensor_mul(sep2[:], gt2[:], sep2[:])
            fus = cs.tile([c.D, c.BS], f32, tag="fus")
            nc.vector.tensor_add(fus[:], sep[:], sep2[:])
            nc.vector.tensor_add(fus[:], fus[:], com[:])
            Xs = cb.tile([c.D, c.BS], f32, tag="Xs")
            nc.vector.scalar_tensor_tensor(Xs[:], fus[:], 1.0 / 3.0, Xit_m[:],
                                           op0=OP.mult, op1=OP.add)
            nc.vector.tensor_add(Xs[:], Xs[:], Xim_m[:])
            nc.vector.tensor_add(Xs[:], Xs[:], Xtx_m[:])

            # layer-1 h-matmuls (independent of the session vector)
            h1raw = cb.tile([c.L2, c.NPAIR * c.D], b16, tag="h1raw")
            for g in range(c.NPAIR // 4):
                phх = psT.tile([128, 512], f32, tag="psTf", name=f"h1p{g}")
                for j in range(4):
                    p = 4 * g + j
                    nc.tensor.matmul(phх[0:c.L2, j * c.D:(j + 1) * c.D],
                                     lhsT=Asb[:, p * c.L2:(p + 1) * c.L2],
                                     rhs=h0[:, p * c.D:(p + 1) * c.D],
                                     start=True, stop=True)
                nc.scalar.copy(h1raw[:, g * 512:(g + 1) * 512],
                               phх[0:c.L2, :])

            # transpose Xs -> XsT [64, 128] bf16
            trx = psT.tile([128, 512], f32, tag="psTf", name="trx")
            nc.tensor.transpose(trx[0:c.BS, 0:c.D], Xs[:], identf[:])
            XsT = cb.tile([c.BS, c.D], b16, tag="XsT")
            nc.vector.tensor_copy(XsT[:], trx[0:c.BS, 0:c.D])

            psT_ctx.__exit__(None, None, None)
            psS_ctx.__exit__(None, None, None)
            psA_ctx.__exit__(None, None, None)

            # ---------------- hypergraph, groups of 4 pairs ----------------
            # h1 = An.T h0 + srep ; h2 = An.T h1 + srep  (An = (Dn^-1 Abar)^T)
            houtf = cb.tile([c.L2, c.NPAIR * c.D], f16, tag="houtf")
            with (
                tc.tile_pool(name="psB", bufs=2, space="PSUM") as psB,
                tc.tile_pool(name="psH", bufs=5, space="PSUM") as psH,
            ):
                for g in range(c.NPAIR // 4):
                    ps_ = [g * 4 + j for j in range(4)]
                    sb = psB.tile([128, 512], f32, tag="psB", name=f"sb{g}")
                    for j, p in enumerate(ps_):
                        nc.tensor.matmul(sb[0:c.L2, j * c.D:(j + 1) * c.D],
                                         lhsT=sel[:, p * c.L2:(p + 1) * c.L2],
                                         rhs=XsT[:], start=True, stop=True)
                    srepS = cs.tile([c.L2, 4 * c.D], f32, tag="srepS")
                    nc.scalar.copy(srepS[:], sb[0:c.L2, :])

                    hh1 = cs.tile([c.L2, 4 * c.D], b16, tag="hh1")
                    for j, p in enumerate(ps_):
                        nc.vector.scalar_tensor_tensor(
                            hh1[:, j * c.D:(j + 1) * c.D],
                            h1raw[:, p * c.D:(p + 1) * c.D],
                            idnA[:, p:p + 1],
                            srepS[:, j * c.D:(j + 1) * c.D],
                            op0=OP.mult, op1=OP.add)

                    ph2 = psH.tile([128, 512], f32, tag="psH", name=f"ph2{g}")
                    for j, p in enumerate(ps_):
                        nc.tensor.matmul(ph2[0:c.L2, j * c.D:(j + 1) * c.D],
                                         lhsT=Asb[:, p * c.L2:(p + 1) * c.L2],
                                         rhs=hh1[:, j * c.D:(j + 1) * c.D],
                                         start=True, stop=True)
                    for j, p in enumerate(ps_):
                        nc.vector.scalar_tensor_tensor(
                            houtf[:, p * c.D:(p + 1) * c.D],
                            ph2[0:c.L2, j * c.D:(j + 1) * c.D],
                            idnA[:, p:p + 1],
                            srepS[:, j * c.D:(j + 1) * c.D],
                            op0=OP.mult, op1=OP.add)
                    nc.scalar.dma_start(
                        outd[:, g * 512:(g + 1) * 512],
                        houtf[:, g * 512:(g + 1) * 512])
    nc.compile()
    return nc


_CACHE = {}


def _get_program(c: Cfg):
    key = (c.N, c.B)
    if key not in _CACHE:
        _CACHE[key] = build_program(c)
    return _CACHE[key]


def _prep_inputs(c: Cfg, inputs, item, mask_item, Hs, emb_table, img_table,
                 txt_table, img_W, img_b, txt_W, txt_b, gate_v_W, gate_v_b,
                 gate_t_W, gate_t_b, qc_W1, qc_b1, qc_W2):
    f32 = np.float32
    inputs = np.asarray(inputs)
    item = np.asarray(item)
    maskf = np.asarray(mask_item).astype(f32)
    Hs = np.asarray(Hs).astype(f32)
    emb_table = np.asarray(emb_table).astype(f32)
    img_table = np.asarray(img_table).astype(f32)
    txt_table = np.asarray(txt_table).astype(f32)
    bcomb = (0.1 * np.asarray(img_b) + 0.15 * np.asarray(txt_b)).astype(f32)

    # Wcat: [0.1*img_W ; 0.15*txt_W ; pad ; I] -> [128, 15, 128] fp8
    Wc = np.zeros((c.KP * 128, c.D), f32)
    Wc[:c.IMG] = 0.1 * np.asarray(img_W)
    Wc[c.IMG:c.IMG + c.TXT] = 0.15 * np.asarray(txt_W)
    Wc[14 * 128:] = np.eye(c.D, dtype=f32)
    Wcatd = np.ascontiguousarray(
        Wc.astype(FP8).reshape(c.KP, 128, c.D).transpose(1, 0, 2)
    ).reshape(128, c.KP * c.D)

    # session weight chunks: rr col layout = emb(0:128) img(128:1128)
    # txt(1128:1896) ind(1896) pad(1897:1920)
    W2 = np.zeros((c.NW, 128, c.D), f32)
    rows = np.arange(128)
    for i, m in enumerate(range(1, 9)):
        gl = m * 128 + rows
        fi = gl - 128
        val = np.where((gl >= 128) & (gl < 1128), 1.0, 0.0)
        W2[i] = np.asarray(img_W)[np.clip(fi, 0, c.IMG - 1)] * val[:, None]
    W2[8, 104] = np.asarray(img_b)
    for i, m in enumerate(range(8, 15)):
        gl = m * 128 + rows
        fi = gl - 1128
        val = np.where((gl >= 1128) & (gl < 1896), 1.0, 0.0)
        W2[9 + i] = np.asarray(txt_W)[np.clip(fi, 0, c.TXT - 1)] * val[:, None]
    W2[16, 104] = np.asarray(txt_b)
    Wsd = np.ascontiguousarray(
        W2.astype(BF16).transpose(1, 0, 2)).reshape(128, c.NW * c.D)

    def gather(tab, ids):
        r = tab[np.maximum(ids - 1, 0)]
        r[ids == 0] = 0.0
        return r

    in_maps = []
    for kk in range(c.NC):
        b0, b1 = kk * c.BS, (kk + 1) * c.BS
        # --- h0 projection inputs (pair-major entry order) ---
        iid = inputs[b0:b1].reshape(c.NPAIR, c.L2).ravel()
        A = np.zeros((c.KP * 128, c.NE), FP8)
        A[:c.IMG] = gather(img_table, iid).T.astype(FP8)
        A[c.IMG:c.IMG + c.TXT] = gather(txt_table, iid).T.astype(FP8)
        ge = gather(emb_table, iid) + bcomb
        ge[iid == 0] = 0.0
        A[14 * 128:] = ge.T.astype(FP8)
        A3 = A.reshape(c.KP, 128, c.NE)
        blocks = [np.ascontiguousarray(
            A3[:, :, r0:r0 + w].transpose(1, 0, 2)).reshape(128, c.KP * w)
            for r0, w in c.RCH]
        prTd = np.concatenate(blocks, axis=1)

        # --- session raw rows (b-major entries, [p][seg][k][512]) ---
        tid = item[b0:b1].ravel()
        R = np.zeros((c.NE, c.RW), f32)
        re_ = gather(emb_table, tid) + bcomb
        re_[tid == 0] = 0.0
        R[:, 0:128] = re_
        R[:, 128:1128] = gather(img_table, tid)
        R[:, 1128:1896] = gather(txt_table, tid)
        R[:, 1896] = (tid > 0).astype(f32)
        rrd = np.ascontiguousarray(
            R.astype(FP8).reshape(c.KR, 128, c.MR, c.D).transpose(1, 2, 0, 3)
        ).reshape(128, c.MR * c.KR * c.D)

        mk = maskf[b0:b1]
        M = np.zeros((c.NE, c.BS), f32)
        M[np.arange(c.NE), np.arange(c.NE) // c.L] = mk.ravel()
        mkTd = np.ascontiguousarray(
            M.astype(FP8).reshape(c.KR, 128, c.BS).transpose(1, 0, 2)
        ).reshape(128, c.KR * c.BS)

        # --- hypergraph blocks ---
        Hk = Hs[b0:b1]
        Gd = np.zeros((c.L2, c.NPAIR, c.L2), f32)
        GTd = np.zeros((c.L2, c.NPAIR, c.L2), f32)
        for p in range(c.NPAIR):
            Gd[:c.L, p, :c.L] = Hk[2 * p]
            Gd[c.L:, p, c.L:] = Hk[2 * p + 1]
            GTd[:c.L, p, :c.L] = Hk[2 * p].T
            GTd[c.L:, p, c.L:] = Hk[2 * p + 1].T
        seldm = np.zeros((c.BS, c.NPAIR, c.L2), f32)
        for p in range(c.NPAIR):
            seldm[2 * p, p, :c.L] = 1.0
            seldm[2 * p + 1, p, c.L:] = 1.0

        in_maps.append({
            "prTd": prTd, "rrd": rrd, "mkTd": mkTd,
            "Wcatd": Wcatd, "Wsd": Wsd,
            "Gd": Gd.astype(FP8).reshape(c.L2, c.NPAIR * c.L2),
            "GTd": GTd.astype(FP8).reshape(c.L2, c.NPAIR * c.L2),
            "seld": seldm.astype(BF16).reshape(c.BS, c.NPAIR * c.L2),
            "mbld": np.ascontiguousarray(mk.T).astype(BF16),
            "gvW": np.asarray(gate_v_W).astype(f32),
            "gvB": np.asarray(gate_v_b).reshape(c.D, 1).astype(f32),
            "gtW": np.asarray(gate_t_W).astype(f32),
            "gtB": np.asarray(gate_t_b).reshape(c.D, 1).astype(f32),
            "q1W": np.asarray(qc_W1).astype(f32),
            "q1B": np.asarray(qc_b1).reshape(c.D, 1).astype(f32),
            "q2W": np.asarray(qc_W2).astype(f32),
        })
    return in_maps


def run(c: Cfg, trace=False, **inputs):
    nc = _get_program(c)
    in_maps = _prep_inputs(c, **{k: np.asarray(v) for k, v in inputs.items()})
    res = bass_utils.run_bass_kernel_spmd(
        nc, in_maps, core_ids=list(range(c.NC)), trace=trace)
    outs = []
    for r in res.results:
        o = np.asarray(r["outd"]).astype(np.float32)
        o = o.reshape(c.L2, c.NPAIR, c.D).transpose(1, 0, 2)
        outs.append(o.reshape(c.NPAIR, 2, c.L, c.D).reshape(c.BS, c.L, c.D))
    out = np.concatenate(outs, axis=0)
    return out.astype(np.float32), res


def kernel(**inputs):
    out, _ = run(REAL, trace=False, **inputs)
    return out
